# revision 66
# baseline (speedup 1.0000x reference)
"""Trainium2 Bass kernel for nn_Block_10024453669245 (dense transformer block).

Strategy (8 NeuronCores):
  - warmup: dummy 32B AllGather prepays collective-communicator init.
  - Phase A: per-core LN1 stats on its 512 own tokens + tiny AllGather.
  - Phase B: QKV tensor-parallel over heads (2 heads/core). fp32r matmuls
    against host-transposed xT. LN1 is folded in: the rank-1 term
    (-colsum x murstd) is added via a K=1 fp32r matmul inside the same
    PSUM accumulation group; eviction is a single DVE mult by rstd.
    Produces qT,kT [d,t] and v [t,d] in bf16, resident in SBUF.
  - Phase C: causal attention head-major, no-max-sub softmax, S^T tiles,
    exp on ACT, causal masks on diagonal tiles, O^T and denominator
    accumulated on PE. Per-head AllToAll (2MB) fires as soon as that
    head's outputs are done, overlapping the other head's attention.
  - Phase E: MLP token-sharded (512 tokens/core) in bf16. ln2's weight is
    folded into w1 (host), the mean term via K=1 matmul fold, rstd2 at
    PSUM eviction. gelu = ACT Gelu_apprx_tanh. Residual from f32 spill.
    Output written transposed [C, 512] per core; host reassembles.
  DMAs are spread over sync/gpsimd/vector queues to avoid serializing.
"""
import sys, math

sys.path.insert(0, "/opt/trn_rl_repo")

import numpy as np
import ml_dtypes

import concourse.bass as bass
import concourse.tile as tile
from concourse import bacc, mybir
from concourse.bass_utils import run_bass_kernel_spmd

# ---------------- constants (hardcoded problem shape) ----------------
P = 128
B, T, C = 2, 2048, 2048
H, D = 16, 128
R = 8                 # cores
HL = H // R           # heads per core
TOK = B * T // R      # own tokens per core
CT = C // P           # 16 c-tiles
NT = T // 512         # 4 t-blocks per batch
M1 = 4 * C            # 8192
MT = M1 // P          # 64 m-tiles
MG = 16               # m-groups of 4 m-tiles (512 cols) for matmul1
EPS = 1e-5
SCALE = 1.0 / math.sqrt(D)

F32 = mybir.dt.float32
F32R = mybir.dt.float32r
BF16 = mybir.dt.bfloat16
F8 = mybir.dt.float8e4
AF = mybir.ActivationFunctionType
ALU = mybir.AluOpType
DR = mybir.MatmulPerfMode.DoubleRow
KP = CT // 2              # 8 fp8 DoubleRow k-pair groups
S8 = 128.0                # fp8 weight scale for q/k/v

_CACHE = {}
DEBUG = False


def _pbc(t, n_free):
    """partition-broadcast AP over a 1-D dram tile view."""
    return bass.AP(tensor=t.tensor, offset=t.offset, ap=[[0, P], [1, n_free]])


def _row(ap1d):
    return ap1d.rearrange("(o t) -> o t", o=1)


def _build():
    nc = bacc.Bacc("TRN2", target_bir_lowering=False, debug=False, num_devices=R)

    # ---------------- I/O ----------------
    # all large inputs are host-arranged so big DMAs are per-partition
    # CONTIGUOUS 2D (strided 1KB-row gathers run at ~32GB/s vs ~350)
    xTb_d = nc.dram_tensor("xTb", [B, NT, P, KP, 2, 512], F8,
                           kind="ExternalInput")
    xT_own_d = nc.dram_tensor("xT_own", [P, CT * TOK], F32,
                              kind="ExternalInput")
    xt_own_d = nc.dram_tensor("xt_own", [P, CT * 512], BF16,
                              kind="ExternalInput")
    wq_d = nc.dram_tensor("wq", [P, KP, 2, HL * D], F8, kind="ExternalInput")
    wk_d = nc.dram_tensor("wk", [P, KP, 2, HL * D], F8, kind="ExternalInput")
    wv_d = nc.dram_tensor("wv", [P, KP, 2, HL * D], F8, kind="ExternalInput")
    nsq_d = nc.dram_tensor("nsq", [HL * D], BF16, kind="ExternalInput")
    nsk_d = nc.dram_tensor("nsk", [HL * D], BF16, kind="ExternalInput")
    nsv_d = nc.dram_tensor("nsv", [HL * D], BF16, kind="ExternalInput")
    w1_d = nc.dram_tensor("w1", [MG, P, CT * 512], BF16, kind="ExternalInput")
    ns1_d = nc.dram_tensor("ns1", [M1], BF16, kind="ExternalInput")
    w2r_d = nc.dram_tensor("w2r", [CT, MT, P, P], BF16, kind="ExternalInput")
    masks_d = nc.dram_tensor("masks", [P, P], BF16, kind="ExternalInput")
    out_d = nc.dram_tensor("outT", [C, TOK], F32, kind="ExternalOutput")

    with tile.TileContext(nc) as tc:
        with tc.tile_pool(name="dram", bufs=1, space="DRAM") as dram, \
             tc.tile_pool(name="psum", bufs=8, space="PSUM") as psum, \
             tc.tile_pool(name="singles", bufs=1) as singles:

            # internal DRAM
            warm_in = dram.tile([8], F32)
            warm_out = dram.tile([R, 8], F32)
            stats_loc = dram.tile([2, TOK], F32)
            stats_g = dram.tile([R, 2, TOK], F32)
            a2a_in = [dram.tile([R, P, 512], BF16, name=f"a2a_in{h}")
                      for h in range(HL)]
            a2a_out = [dram.tile([R, P, 512], BF16, name=f"a2a_out{h}")
                       for h in range(HL)]
            mlp_stat_b = dram.tile([2, TOK], F32)

            def ps():
                return psum.tile([P, 512], F32, tag="ps", name="ps")

            # warmup collective: pays communicator init while phase A runs
            nc.gpsimd.collective_compute(
                "AllGather", ALU.bypass, replica_groups=[list(range(R))],
                ins=[warm_in.opt()], outs=[warm_out.opt()])

            # small constants
            eps_t = singles.tile([P, 1], F32)
            nc.vector.memset(eps_t, EPS)
            ones_bf = singles.tile([P, 1], BF16)
            nc.vector.memset(ones_bf, 1.0)
            ones_f32 = singles.tile([P, 1], F32)
            nc.vector.memset(ones_f32, 1.0)
            ones_row = singles.tile([1, P], BF16)
            nc.vector.memset(ones_row, 1.0)

            # =========== Phase B+C pools (opened early: weight DMAs
            # go out on three parallel queues before phase A traffic) =======
            _wpool_cm = tc.tile_pool(name="wqkv", bufs=1)
            wpool = _wpool_cm.__enter__()
            wq_t = wpool.tile([P, KP, 2, HL * D], F8)
            wk_t = wpool.tile([P, KP, 2, HL * D], F8)
            wv_t = wpool.tile([P, KP, 2, HL * D], F8)
            nsq_t = wpool.tile([1, HL * D], BF16)
            nc.gpsimd.dma_start(nsq_t, _row(nsq_d.ap()))
            nsk_t = wpool.tile([1, HL * D], BF16)
            nc.gpsimd.dma_start(nsk_t, _row(nsk_d.ap()))
            nsv_t = wpool.tile([1, HL * D], BF16)
            nc.gpsimd.dma_start(nsv_t, _row(nsv_d.ap()))

            # =========== Phase A: LN1 stats on own tokens (from xt_own via
            # PE ones-matmuls). All DMAs batched (one big transfer each):
            # per-dma_start queue overhead is ~1.5us, so many small DMAs
            # starve the pipeline. ========
            with tc.tile_pool(name="stA", bufs=1) as stA:
                pmu0 = ps()
                psq0 = ps()
                xo_blk = stA.tile([P, CT, 512], BF16, name="xo_blk")
                nc.sync.dma_start(xo_blk, xt_own_d.ap())
                nc.scalar.dma_start(wq_t, wq_d.ap())
                nc.gpsimd.dma_start(wk_t, wk_d.ap())
                nc.scalar.dma_start(wv_t, wv_d.ap())
                for ko in range(CT):
                    sqx = stA.tile([P, 512], BF16, tag="sqx", name="sqx", bufs=4)
                    nc.vector.tensor_tensor(sqx, xo_blk[:, ko], xo_blk[:, ko],
                                            ALU.mult)
                    nc.tensor.matmul(pmu0[0:1, :], ones_bf, xo_blk[:, ko],
                                     start=(ko == 0), stop=(ko == CT - 1))
                    nc.tensor.matmul(psq0[0:1, :], ones_bf, sqx,
                                     start=(ko == 0), stop=(ko == CT - 1))
                muA = stA.tile([1, 512], F32, tag="muA", name="muA")
                nc.vector.tensor_scalar(muA, pmu0[0:1, :], 1.0 / C, None, ALU.mult)
                varA = stA.tile([1, 512], F32, tag="varA", name="varA")
                nc.vector.tensor_scalar(varA, psq0[0:1, :], 1.0 / C, None, ALU.mult)
                musqA = stA.tile([1, 512], F32, tag="musqA", name="musqA")
                nc.vector.tensor_tensor(musqA, muA, muA, ALU.mult)
                nc.vector.tensor_tensor(varA, varA, musqA, ALU.subtract)
                rstdA = stA.tile([1, 512], F32, tag="rstdA", name="rstdA")
                nc.scalar.activation(rstdA, varA, AF.Sqrt, bias=eps_t[0:1])
                nc.vector.reciprocal_approx_fast(out=rstdA, in_=rstdA)
                murstdA = stA.tile([1, 512], F32, tag="murstdA", name="murstdA")
                nc.vector.tensor_tensor(murstdA, muA, rstdA, ALU.mult)
                # stats_g[.,0] carries rstd/S8: the fp8 qkv psums are in
                # S8-scaled units, one evict multiply undoes both
                rstdA_s = stA.tile([1, 512], F32, tag="rstdAs", name="rstdA_s")
                nc.vector.tensor_scalar(rstdA_s, rstdA, 1.0 / S8, None,
                                        ALU.mult)
                nc.gpsimd.dma_start(_row(stats_loc[0, :]), rstdA_s)
                nc.gpsimd.dma_start(_row(stats_loc[1, :]), murstdA)
            nc.gpsimd.collective_compute(
                "AllGather", ALU.bypass,
                replica_groups=[list(range(R))],
                ins=[stats_loc.opt()], outs=[stats_g.opt()])
            masks_t = wpool.tile([P, P], BF16)
            nc.scalar.dma_start(masks_t, masks_d.ap())

            # =========== Phase B+C pools ===========
            with tc.tile_pool(name="qkvres", bufs=1) as qkvres, \
                 tc.tile_pool(name="xtp", bufs=2) as xtp, \
                 tc.tile_pool(name="reps", bufs=3) as reps, \
                 tc.tile_pool(name="tmps", bufs=4) as tmps, \
                 tc.tile_pool(name="attn", bufs=5) as attnp:

                # persistent qkv (bf16)
                qT = [[qkvres.tile([P, T], BF16, name=f"qT{h}{b}")
                       for b in range(B)] for h in range(HL)]
                kT = [[qkvres.tile([P, T], BF16, name=f"kT{h}{b}")
                       for b in range(B)] for h in range(HL)]
                vsb = [qkvres.tile([P, T // P, HL * D], BF16, name=f"v{b}")
                       for b in range(B)]

                # =========== Phase B: QKV ===========
                # first RAW_BLOCKS blocks evict uncorrected (stats AG not yet
                # done); LN1 correction applied in-place afterwards.
                RAW_BLOCKS = 4   # the b=1 blocks, processed first
                sv_rep_b = wpool.tile([P, HL * D], BF16)
                nc.gpsimd.dma_start(sv_rep_b, bass.AP(
                    tensor=nsv_d, offset=0, ap=[[0, P], [1, HL * D]]))
                sv_rep = wpool.tile([P, HL * D], F32)
                nc.vector.tensor_copy(sv_rep, sv_rep_b)
                nsq_cb = wpool.tile([P, HL], BF16)
                nc.gpsimd.dma_start(nsq_cb, nsq_d.ap().rearrange("(hl d) -> d hl", d=P))
                nsq_c = wpool.tile([P, HL], F32)
                nc.vector.tensor_copy(nsq_c, nsq_cb)
                nsk_cb = wpool.tile([P, HL], BF16)
                nc.gpsimd.dma_start(nsk_cb, nsk_d.ap().rearrange("(hl d) -> d hl", d=P))
                nsk_c = wpool.tile([P, HL], F32)
                nc.vector.tensor_copy(nsk_c, nsk_cb)
                for b in range(B):
                    for tb in range(NT):
                        j = NT * b + tb
                        t0 = 512 * tb
                        raw = j < RAW_BLOCKS
                        if not raw:
                            # these depend on the stats AllGather — emitting
                            # them for raw blocks would stall the in-order
                            # gpsimd DMA queue on the collective
                            murow_f = reps.tile([1, 512], F32, tag="murowf",
                                                name="murow_f")
                            nc.gpsimd.dma_start(murow_f, _row(stats_g[j, 1, :]))
                            murow = reps.tile([1, 512], BF16, tag="murow",
                                              name="murow")
                            nc.vector.tensor_copy(murow, murow_f)
                            rstd_rep = reps.tile([P, 512], F32, tag="rrep",
                                                 name="rstd_rep")
                            nc.gpsimd.dma_start(rstd_rep, _pbc(stats_g[j, 0, :], 512))

                        pq = [ps() for _ in range(HL)]
                        pk = [ps() for _ in range(HL)]
                        # one bank per 128-token v subtile (start=True clears
                        # the whole bank, chains must not share one)
                        pv = [ps() for _ in range(4)]
                        # one 2MB batched DMA per t-block (per-DMA overhead
                        # dominates many small loads)
                        xt_blk = xtp.tile([P, KP, 2, 512], F8, tag="xt",
                                          name="xt", bufs=3)
                        (nc.sync if j % 2 == 0 else nc.scalar).dma_start(
                            xt_blk, xTb_d.ap()[b, tb])
                        for kp in range(KP):
                            xt = xt_blk[:, kp]          # [P, 2, 512]
                            st_flag = kp == 0
                            lst = raw and kp == KP - 1
                            for hl in range(HL):
                                nc.tensor.matmul(
                                    pq[hl],
                                    wq_t[:, kp, :, hl * D:(hl + 1) * D], xt,
                                    start=st_flag, stop=lst, perf_mode=DR)
                                nc.tensor.matmul(
                                    pk[hl],
                                    wk_t[:, kp, :, hl * D:(hl + 1) * D], xt,
                                    start=st_flag, stop=lst, perf_mode=DR)
                            for ss in range(4):
                                nc.tensor.matmul(
                                    pv[ss][:, 0:256],
                                    xt[:, :, ss * P:(ss + 1) * P],
                                    wv_t[:, kp],
                                    start=st_flag, stop=lst, perf_mode=DR)
                        if raw:
                            for hl in range(HL):
                                nc.vector.tensor_copy(
                                    qT[hl][b][:, t0:t0 + 512], pq[hl])
                                nc.vector.tensor_copy(
                                    kT[hl][b][:, t0:t0 + 512], pk[hl])
                            for ss in range(4):
                                nc.vector.tensor_copy(
                                    vsb[b][:, tb * 4 + ss, :], pv[ss][:, 0:256])
                            continue
                        # rank-1 LN fold: += (-colsum) x murstd  (K=1 matmul)
                        for hl in range(HL):
                            nc.tensor.matmul(
                                pq[hl], nsq_t[0:1, hl * D:(hl + 1) * D], murow,
                                start=False, stop=True)
                            nc.tensor.matmul(
                                pk[hl], nsk_t[0:1, hl * D:(hl + 1) * D], murow,
                                start=False, stop=True)
                        for ss in range(4):
                            nc.tensor.matmul(
                                pv[ss][:, 0:256],
                                murow[0:1, ss * P:(ss + 1) * P], nsv_t,
                                start=False, stop=True)
                        # evictions: single mult by rstd
                        for hl in range(HL):
                            nc.vector.tensor_tensor(
                                qT[hl][b][:, t0:t0 + 512], pq[hl], rstd_rep, ALU.mult)
                            nc.vector.tensor_tensor(
                                kT[hl][b][:, t0:t0 + 512], pk[hl], rstd_rep, ALU.mult)
                        for ss in range(4):
                            si = tb * 4 + ss
                            rstd_c = tmps.tile([P, 1], F32, tag="rc", name="rc")
                            nc.gpsimd.dma_start(
                                rstd_c,
                                stats_g[j, 0, ss * P:(ss + 1) * P]
                                .rearrange("(p o) -> p o", o=1))
                            nc.vector.tensor_scalar(
                                vsb[b][:, si, :], pv[ss][:, 0:256], rstd_c, None,
                                ALU.mult)

                # in-place LN1 fixups for raw blocks (wait on stats AG).
                # They run on GPSIMD: on the DVE queue they head-block
                # phase C's mask ops for ~35us (in-order queues).
                for j in range(RAW_BLOCKS):
                    fb, ftb = j // NT, j % NT
                    ft0 = 512 * ftb
                    fmur = reps.tile([P, 512], F32, tag="fxmr", name="fx_mur")
                    nc.gpsimd.dma_start(fmur, _pbc(stats_g[j, 1, :], 512))
                    frstd = reps.tile([P, 512], F32, tag="fxrr", name="fx_rstd")
                    nc.gpsimd.dma_start(frstd, _pbc(stats_g[j, 0, :], 512))
                    for hl in range(HL):
                        for (tile_, s_col) in ((qT[hl][fb], nsq_c[:, hl:hl + 1]),
                                               (kT[hl][fb], nsk_c[:, hl:hl + 1])):
                            tmp = tmps.tile([P, 512], F32, tag="fxt", name="fxt")
                            nc.gpsimd.tensor_scalar(tmp, fmur, s_col, None,
                                                    ALU.mult)
                            nc.gpsimd.tensor_tensor(tmp, tile_[:, ft0:ft0 + 512],
                                                    tmp, ALU.add)
                            nc.gpsimd.tensor_tensor(tile_[:, ft0:ft0 + 512], tmp,
                                                    frstd, ALU.mult)
                    for ss in range(4):
                        si = ftb * 4 + ss
                        frc = tmps.tile([P, 1], F32, tag="rc", name="rc")
                        nc.gpsimd.dma_start(
                            frc, stats_g[j, 0, ss * P:(ss + 1) * P]
                            .rearrange("(p o) -> p o", o=1))
                        fmc = tmps.tile([P, 1], F32, tag="mc", name="mc")
                        nc.gpsimd.dma_start(
                            fmc, stats_g[j, 1, ss * P:(ss + 1) * P]
                            .rearrange("(p o) -> p o", o=1))
                        tmpv = tmps.tile([P, HL * D], F32, tag="fxv", name="fxv")
                        nc.gpsimd.tensor_scalar(tmpv, sv_rep, fmc, None, ALU.mult)
                        nc.gpsimd.tensor_tensor(tmpv, vsb[fb][:, si, :], tmpv,
                                                ALU.add)
                        nc.gpsimd.tensor_scalar(vsb[fb][:, si, :], tmpv, frc,
                                                None, ALU.mult)

                # =========== Phase C: attention (head-major) ===========
                # b=1 first: b=0's raw-block LN fixups (DVE) then overlap
                # with (hl0,b1) attention instead of stalling its start.
                for hl in range(HL):
                    for b in (1, 0):
                        for tb in range(NT):
                            t0 = 512 * tb
                            n_s = 4 * (tb + 1)
                            pot = ps()
                            pden = ps()
                            for si in range(n_s):
                                m = si - (n_s - 4)
                                w0 = max(m, 0) * P   # masked-out prefix width
                                pS = ps()
                                nc.tensor.matmul(
                                    pS[:, w0:512],
                                    kT[hl][b][:, si * P:(si + 1) * P],
                                    qT[hl][b][:, t0 + w0:t0 + 512],
                                    start=True, stop=True)
                                pt = attnp.tile([P, 512], BF16, tag="pt", name="pt")
                                nc.scalar.activation(pt[:, w0:512], pS[:, w0:512],
                                                     AF.Exp)
                                if m >= 0:
                                    nc.vector.tensor_tensor(
                                        pt[:, w0:w0 + P], pt[:, w0:w0 + P],
                                        masks_t, ALU.mult)
                                nc.tensor.matmul(
                                    pot[:, w0:512],
                                    vsb[b][:, si, hl * D:(hl + 1) * D],
                                    pt[:, w0:512],
                                    start=(si == 0), stop=(si == n_s - 1))
                                nc.tensor.matmul(
                                    pden[0:1, w0:512], ones_bf, pt[:, w0:512],
                                    start=(si == 0), stop=(si == n_s - 1))
                            # normalize: OT / den
                            den_r = attnp.tile([1, 512], F32, tag="dr", name="den_r")
                            nc.vector.reciprocal_approx_fast(out=den_r, in_=pden[0:1, :])
                            # den roundtrip + a2a staging on the idle sync
                            # queue (gpsimd is busy with the LN fixups)
                            den_d = dram.tile([512], F32, tag="den_d", bufs=4,
                                              name="den_d")
                            nc.sync.dma_start(_row(den_d), den_r)
                            den_rep = reps.tile([P, 512], F32, tag="denrep",
                                                name="den_rep")
                            nc.sync.dma_start(den_rep, _pbc(den_d, 512))
                            ot = attnp.tile([P, 512], BF16, tag="ot", name="ot")
                            nc.vector.tensor_tensor(ot, pot, den_rep, ALU.mult)
                            nc.sync.dma_start(a2a_in[hl][NT * b + tb, :, :], ot)
                    # per-head AllToAll fires as soon as head hl is done
                    nc.gpsimd.collective_compute(
                        "AllToAll", ALU.bypass,
                        replica_groups=[list(range(R))],
                        ins=[a2a_in[hl].opt()], outs=[a2a_out[hl].opt()])

            _wpool_cm.__exit__(None, None, None)

            # =========== Phase E: MLP (token-sharded, bf16) ===========
            with tc.tile_pool(name="mlp_x1", bufs=1) as x1p, \
                 tc.tile_pool(name="mlp_sq", bufs=2) as sqp, \
                 tc.tile_pool(name="mlp_x1bf", bufs=1) as x1bfp, \
                 tc.tile_pool(name="mlp_g", bufs=1) as gp, \
                 tc.tile_pool(name="mlp_w1", bufs=2) as w1p, \
                 tc.tile_pool(name="mlp_w2", bufs=2) as w2p, \
                 tc.tile_pool(name="mlp_z", bufs=4) as zp, \
                 tc.tile_pool(name="mlp_out", bufs=2) as outp:

                x1bf = [x1bfp.tile([P, TOK], BF16, name=f"x1bf{i}")
                        for i in range(CT)]
                # pass 1: build x1 tiles (evens first: only need a2a head 0),
                # stats matmuls, bf16 copy. x1 stays RESIDENT in SBUF (no
                # DRAM spill). All DMAs batched. mm1's mg0 even-ct matmuls
                # are emitted between the even and odd builds so PE overlaps
                # the head-1 AllToAll.
                pmu = ps()
                psq = ps()
                order = [2 * i for i in range(CT // 2)] + \
                        [2 * i + 1 for i in range(CT // 2)]
                evens = order[:CT // 2]
                odds = order[CT // 2:]

                # x1 is built IN PLACE over xo_all (residual add) and stays
                # resident in SBUF through mm2 (no DRAM spill/readback)
                x1_all = x1p.tile([P, CT, TOK], F32, name="x1_all")
                nc.sync.dma_start(x1_all, xT_own_d.ap())

                def build_x1(idx, ct):
                    at = x1p.tile([P, TOK], BF16, tag="at", name="at", bufs=3)
                    nc.gpsimd.dma_start(at, a2a_out[ct % 2][ct // 2])
                    nc.vector.tensor_tensor(x1_all[:, ct], x1_all[:, ct],
                                            at, ALU.add)
                    nc.vector.tensor_copy(x1bf[ct], x1_all[:, ct])
                    sq2 = sqp.tile([P, TOK], BF16, tag="sq2", name="sq2")
                    nc.vector.tensor_tensor(sq2, x1bf[ct], x1bf[ct], ALU.mult)
                    nc.tensor.matmul(pmu[0:1, :], ones_bf, x1bf[ct],
                                     start=(idx == 0), stop=(idx == CT - 1))
                    nc.tensor.matmul(psq[0:1, :], ones_bf, sq2,
                                     start=(idx == 0), stop=(idx == CT - 1))

                def mm1_w1_tiles(mg):
                    w1blk = w1p.tile([P, CT, 512], BF16, tag="w1t", name="w1t")
                    nc.sync.dma_start(w1blk, w1_d.ap()[mg])
                    ns1g = zp.tile([1, 512], BF16, tag="ns1g", name="ns1g")
                    nc.gpsimd.dma_start(
                        ns1g, _row(ns1_d.ap()[mg * 512:(mg + 1) * 512]))
                    return w1blk, ns1g

                def mm1_group(mg, blk, pg, group, first):
                    for ci, ct in enumerate(group):
                        st_f = first and ci == 0
                        for ml in range(4):
                            nc.tensor.matmul(
                                pg[ml], blk[:, ct, ml * P:(ml + 1) * P],
                                x1bf[ct], start=st_f, stop=False)

                def mm1_finish(mg, ns1g, pg):
                    for ml in range(4):
                        mt = mg * 4 + ml
                        nc.tensor.matmul(
                            pg[ml], ns1g[0:1, ml * P:(ml + 1) * P],
                            murow2, start=False, stop=True)
                        zt = zp.tile([P, TOK], BF16, tag="zt", name="zt")
                        nc.vector.tensor_tensor(zt, pg[ml], rstd2_rep,
                                                ALU.mult)
                        nc.scalar.activation(gT[:, mt, :], zt,
                                             AF.Gelu_apprx_tanh)

                gT = gp.tile([P, MT, TOK], BF16)
                for idx, ct in enumerate(evens):
                    build_x1(idx, ct)
                # mg0 evens: runs on PE while head-1 A2A is in flight
                # (psum: pmu+psq+4 banks = 6 <= 8, no deadlock)
                wts0, ns1g0 = mm1_w1_tiles(0)
                pg0 = [ps() for _ in range(4)]
                mm1_group(0, wts0, pg0, evens, True)
                for idx, ct in enumerate(odds):
                    build_x1(CT // 2 + idx, ct)
                # finalize stats: mu = pmu/C ; var = psq/C - mu^2
                mu2 = singles.tile([1, TOK], F32)
                nc.vector.tensor_scalar(mu2, pmu[0:1, :], 1.0 / C, None, ALU.mult)
                var2 = singles.tile([1, TOK], F32)
                nc.vector.tensor_scalar(var2, psq[0:1, :], 1.0 / C, None, ALU.mult)
                musq = singles.tile([1, TOK], F32)
                nc.vector.tensor_tensor(musq, mu2, mu2, ALU.mult)
                nc.vector.tensor_tensor(var2, var2, musq, ALU.subtract)
                rstd2 = singles.tile([1, TOK], F32)
                nc.scalar.activation(rstd2, var2, AF.Sqrt, bias=eps_t[0:1])
                nc.vector.reciprocal_approx_fast(out=rstd2, in_=rstd2)
                nc.gpsimd.dma_start(_row(mlp_stat_b[0, :]), mu2)
                nc.gpsimd.dma_start(_row(mlp_stat_b[1, :]), rstd2)
                murow2 = singles.tile([1, TOK], BF16)
                nc.vector.tensor_copy(murow2, mu2)
                rstd2_rep = singles.tile([P, TOK], F32)
                nc.gpsimd.dma_start(rstd2_rep, _pbc(mlp_stat_b[1, :], TOK))

                # mg0 odds + fold + evict
                mm1_group(0, wts0, pg0, odds, False)
                mm1_finish(0, ns1g0, pg0)
                # remaining 15 mgs: 7 pairs + final single
                rest = list(range(1, MG))
                pairs = [rest[i:i + 2] for i in range(0, len(rest), 2)]
                for mgs in pairs:
                    wtss, ns1gs, pgs = {}, {}, {}
                    for mg in mgs:
                        wtss[mg], ns1gs[mg] = mm1_w1_tiles(mg)
                        pgs[mg] = [ps() for _ in range(4)]
                    for group, first in ((evens, True), (odds, False)):
                        for mg in mgs:
                            mm1_group(mg, wtss[mg], pgs[mg], group, first)
                    for mg in mgs:
                        mm1_finish(mg, ns1gs[mg], pgs[mg])

                # matmul2 + residual (x1 resident in SBUF) -> outT
                # w2 streamed in half-co chunks to halve SBUF footprint
                for co in range(CT):
                    w2r_co = w2r_d.ap()[co].rearrange("mo p c -> p mo c")
                    w2t = [w2p.tile([P, MT // 2, P], BF16, tag="w2t",
                                    name="w2t") for _ in range(2)]
                    nc.scalar.dma_start(w2t[0], w2r_co[:, 0:MT // 2])
                    nc.scalar.dma_start(w2t[1], w2r_co[:, MT // 2:MT])
                    po = ps()
                    for mt in range(MT):
                        nc.tensor.matmul(po, w2t[mt // (MT // 2)]
                                         [:, mt % (MT // 2), :], gT[:, mt, :],
                                         start=(mt == 0), stop=(mt == MT - 1))
                    ot2 = outp.tile([P, TOK], F32, tag="ot2", name="ot2")
                    nc.vector.tensor_tensor(ot2, po, x1_all[:, co], ALU.add)
                    nc.scalar.dma_start(out_d.ap()[co * P:(co + 1) * P, :], ot2)

    nc.compile()
    return nc


def _host_prep(x, w_qkv, w1, w2, ln_w):
    x = np.asarray(x, dtype=np.float32)
    w_qkv = np.asarray(w_qkv, dtype=np.float32)
    w1 = np.asarray(w1, dtype=np.float32)
    w2 = np.asarray(w2, dtype=np.float32)
    ln_w = np.asarray(ln_w, dtype=np.float32)

    xT = np.ascontiguousarray(x.transpose(0, 2, 1))            # [B, C, T]
    xT_bf = xT.astype(ml_dtypes.bfloat16)

    def q8(a):
        return np.clip(a, -240, 240).astype(ml_dtypes.float8_e4m3fn)

    # qkv weights quantized to fp8 at scale S8; column sums taken over the
    # QUANTIZED values so the mean-fold cancels exactly
    Wq8 = q8((ln_w[:, None] * w_qkv[:, 0 * C:1 * C]) * (SCALE * S8))
    Wk8 = q8(ln_w[:, None] * w_qkv[:, 1 * C:2 * C] * S8)
    Wv8 = q8(ln_w[:, None] * w_qkv[:, 2 * C:3 * C] * S8)
    nsq_full = -Wq8.astype(np.float64).sum(0).astype(np.float32)
    nsk_full = -Wk8.astype(np.float64).sum(0).astype(np.float32)
    nsv_full = -Wv8.astype(np.float64).sum(0).astype(np.float32)

    def pair_major(w8):
        # [C, n] -> [P, KP, 2, n] (fp8 DoubleRow k-pair layout)
        n = w8.shape[1]
        return np.ascontiguousarray(
            w8.reshape(KP, 2, P, n).transpose(2, 0, 1, 3))

    # xT8: [B, NT, P, KP, 2, 512]
    xT8 = np.ascontiguousarray(
        q8(xT).reshape(B, KP, 2, P, NT, 512).transpose(0, 4, 3, 1, 2, 5))

    w1s = ln_w[:, None] * w1
    # w1 arranged [MG, P, CT*512]: per-mg 2D-contiguous DMA
    w1_bf = np.ascontiguousarray(
        w1s.reshape(CT, P, MG, 512).transpose(2, 1, 0, 3)
    ).reshape(MG, P, CT * 512).astype(ml_dtypes.bfloat16)
    ns1 = -w1s.sum(0, dtype=np.float64).astype(np.float32)
    # w2 reordered: [CT, MT, P(m), P(c)]
    w2r = np.ascontiguousarray(
        w2.reshape(MT, P, CT, P).transpose(2, 0, 1, 3)).astype(ml_dtypes.bfloat16)

    masks = (np.arange(P)[None, :] >= np.arange(P)[:, None]).astype(
        np.float32).astype(ml_dtypes.bfloat16)

    def chan_major(a):
        # [C, N] -> [P, CT*N]: per-partition contiguous rows
        n = a.shape[1]
        return np.ascontiguousarray(
            a.reshape(CT, P, n).transpose(1, 0, 2)).reshape(P, CT * n)

    in_maps = []
    for r in range(R):
        cs = slice(256 * r, 256 * (r + 1))
        b_own, tb_own = r // NT, r % NT
        in_maps.append({
            "xTb": xT8,
            "xT_own": chan_major(np.ascontiguousarray(
                xT[b_own][:, 512 * tb_own: 512 * (tb_own + 1)])),
            "xt_own": chan_major(np.ascontiguousarray(
                xT_bf[b_own][:, 512 * tb_own: 512 * (tb_own + 1)])),
            "wq": pair_major(Wq8[:, cs]),
            "wk": pair_major(Wk8[:, cs]),
            "wv": pair_major(Wv8[:, cs]),
            "nsq": np.ascontiguousarray(nsq_full[cs]).astype(ml_dtypes.bfloat16),
            "nsk": np.ascontiguousarray(nsk_full[cs]).astype(ml_dtypes.bfloat16),
            "nsv": np.ascontiguousarray(nsv_full[cs]).astype(ml_dtypes.bfloat16),
            "w1": w1_bf,
            "ns1": ns1.astype(ml_dtypes.bfloat16),
            "w2r": w2r,
            "masks": masks,
        })
    return in_maps


def get_nc():
    if "nc" not in _CACHE:
        _CACHE["nc"] = _build()
    return _CACHE["nc"]


def run(in_maps, **kw):
    nc = get_nc()
    return run_bass_kernel_spmd(nc, in_maps, core_ids=list(range(R)), **kw)


def kernel(x, w_qkv, w1, w2, ln_w, **kw_unused):
    in_maps = _host_prep(x, w_qkv, w1, w2, ln_w)
    res = run(in_maps)
    out_flat = np.empty((B * T, C), np.float32)
    for r in range(R):
        out_flat[TOK * r: TOK * (r + 1)] = res.results[r]["outT"].T
    return out_flat.reshape(B, T, C)



# revision 68
# speedup vs baseline: 1.2099x; 1.2099x over previous
"""Trainium2 Bass kernel for nn_Block_10024453669245 (dense transformer block).

Strategy (8 NeuronCores):
  - warmup: dummy 32B AllGather prepays collective-communicator init.
  - Phase A: per-core LN1 stats on its 512 own tokens + tiny AllGather.
  - Phase B: QKV tensor-parallel over heads (2 heads/core). fp32r matmuls
    against host-transposed xT. LN1 is folded in: the rank-1 term
    (-colsum x murstd) is added via a K=1 fp32r matmul inside the same
    PSUM accumulation group; eviction is a single DVE mult by rstd.
    Produces qT,kT [d,t] and v [t,d] in bf16, resident in SBUF.
  - Phase C: causal attention head-major, no-max-sub softmax, S^T tiles,
    exp on ACT, causal masks on diagonal tiles, O^T and denominator
    accumulated on PE. Per-head AllToAll (2MB) fires as soon as that
    head's outputs are done, overlapping the other head's attention.
  - Phase E: MLP token-sharded (512 tokens/core) in bf16. ln2's weight is
    folded into w1 (host), the mean term via K=1 matmul fold, rstd2 at
    PSUM eviction. gelu = ACT Gelu_apprx_tanh. Residual from f32 spill.
    Output written transposed [C, 512] per core; host reassembles.
  DMAs are spread over sync/gpsimd/vector queues to avoid serializing.
"""
import sys, math

sys.path.insert(0, "/opt/trn_rl_repo")

import numpy as np
import ml_dtypes

import concourse.bass as bass
import concourse.tile as tile
from concourse import bacc, mybir
from concourse.bass_utils import run_bass_kernel_spmd

# ---------------- constants (hardcoded problem shape) ----------------
P = 128
B, T, C = 2, 2048, 2048
H, D = 16, 128
R = 8                 # cores
HL = H // R           # heads per core
TOK = B * T // R      # own tokens per core
CT = C // P           # 16 c-tiles
NT = T // 512         # 4 t-blocks per batch
M1 = 4 * C            # 8192
MT = M1 // P          # 64 m-tiles
MG = 16               # m-groups of 4 m-tiles (512 cols) for matmul1
EPS = 1e-5
SCALE = 1.0 / math.sqrt(D)

F32 = mybir.dt.float32
F32R = mybir.dt.float32r
BF16 = mybir.dt.bfloat16
F8 = mybir.dt.float8e4
AF = mybir.ActivationFunctionType
ALU = mybir.AluOpType
DR = mybir.MatmulPerfMode.DoubleRow
KP = CT // 2              # 8 fp8 DoubleRow k-pair groups
S8 = 128.0                # fp8 weight scale for q/k/v

_CACHE = {}
DEBUG = False


def _pbc(t, n_free):
    """partition-broadcast AP over a 1-D dram tile view."""
    return bass.AP(tensor=t.tensor, offset=t.offset, ap=[[0, P], [1, n_free]])


def _row(ap1d):
    return ap1d.rearrange("(o t) -> o t", o=1)


def _build():
    nc = bacc.Bacc("TRN2", target_bir_lowering=False, debug=False, num_devices=R)

    # ---------------- I/O ----------------
    # all large inputs are host-arranged so big DMAs are per-partition
    # CONTIGUOUS 2D (strided 1KB-row gathers run at ~32GB/s vs ~350)
    xTb_d = nc.dram_tensor("xTb", [B, NT, P, KP, 2, 512], F8,
                           kind="ExternalInput")
    xT_own_d = nc.dram_tensor("xT_own", [P, CT * TOK], F32,
                              kind="ExternalInput")
    xt_own_d = nc.dram_tensor("xt_own", [P, CT * 512], BF16,
                              kind="ExternalInput")
    wq_d = nc.dram_tensor("wq", [P, KP, 2, HL * D], F8, kind="ExternalInput")
    wk_d = nc.dram_tensor("wk", [P, KP, 2, HL * D], F8, kind="ExternalInput")
    wv_d = nc.dram_tensor("wv", [P, KP, 2, HL * D], F8, kind="ExternalInput")
    nsq_d = nc.dram_tensor("nsq", [HL * D], BF16, kind="ExternalInput")
    nsk_d = nc.dram_tensor("nsk", [HL * D], BF16, kind="ExternalInput")
    nsv_d = nc.dram_tensor("nsv", [HL * D], BF16, kind="ExternalInput")
    w1_d = nc.dram_tensor("w1", [MG, P, CT * 512], BF16, kind="ExternalInput")
    ns1_d = nc.dram_tensor("ns1", [M1], BF16, kind="ExternalInput")
    w2r_d = nc.dram_tensor("w2r", [CT, MT, P, P], BF16, kind="ExternalInput")
    masks_d = nc.dram_tensor("masks", [P, P], BF16, kind="ExternalInput")
    out_d = nc.dram_tensor("outT", [C, TOK], F32, kind="ExternalOutput")

    with tile.TileContext(nc) as tc:
        with tc.tile_pool(name="dram", bufs=1, space="DRAM") as dram, \
             tc.tile_pool(name="psum", bufs=8, space="PSUM") as psum, \
             tc.tile_pool(name="singles", bufs=1) as singles:

            # internal DRAM
            warm_in = dram.tile([8], F32)
            warm_out = dram.tile([R, 8], F32)
            stats_loc = dram.tile([2, TOK], F32)
            stats_g = dram.tile([R, 2, TOK], F32)
            a2a_in = [dram.tile([R, P, 512], BF16, name=f"a2a_in{h}")
                      for h in range(HL)]
            a2a_out = [dram.tile([R, P, 512], BF16, name=f"a2a_out{h}")
                       for h in range(HL)]
            mlp_stat_b = dram.tile([2, TOK], F32)

            def ps():
                return psum.tile([P, 512], F32, tag="ps", name="ps")

            # warmup collective: pays communicator init while phase A runs
            nc.gpsimd.collective_compute(
                "AllGather", ALU.bypass, replica_groups=[list(range(R))],
                ins=[warm_in.opt()], outs=[warm_out.opt()])

            # small constants
            eps_t = singles.tile([P, 1], F32)
            nc.vector.memset(eps_t, EPS)
            ones_bf = singles.tile([P, 1], BF16)
            nc.vector.memset(ones_bf, 1.0)
            ones_f32 = singles.tile([P, 1], F32)
            nc.vector.memset(ones_f32, 1.0)
            ones_row = singles.tile([1, P], BF16)
            nc.vector.memset(ones_row, 1.0)

            # =========== Phase B+C pools (opened early: weight DMAs
            # go out on three parallel queues before phase A traffic) =======
            _wpool_cm = tc.tile_pool(name="wqkv", bufs=1)
            wpool = _wpool_cm.__enter__()
            wq_t = wpool.tile([P, KP, 2, HL * D], F8)
            wk_t = wpool.tile([P, KP, 2, HL * D], F8)
            wv_t = wpool.tile([P, KP, 2, HL * D], F8)
            nsq_t = wpool.tile([1, HL * D], BF16)
            nc.gpsimd.dma_start(nsq_t, _row(nsq_d.ap()))
            nsk_t = wpool.tile([1, HL * D], BF16)
            nc.gpsimd.dma_start(nsk_t, _row(nsk_d.ap()))
            nsv_t = wpool.tile([1, HL * D], BF16)
            nc.gpsimd.dma_start(nsv_t, _row(nsv_d.ap()))

            # =========== Phase A: LN1 stats on own tokens (from xt_own via
            # PE ones-matmuls). All DMAs batched (one big transfer each):
            # per-dma_start queue overhead is ~1.5us, so many small DMAs
            # starve the pipeline. ========
            with tc.tile_pool(name="stA", bufs=1) as stA:
                pmu0 = ps()
                psq0 = ps()
                xo_blk = stA.tile([P, CT, 512], BF16, name="xo_blk")
                nc.sync.dma_start(xo_blk, xt_own_d.ap())
                nc.scalar.dma_start(wq_t, wq_d.ap())
                nc.gpsimd.dma_start(wk_t, wk_d.ap())
                nc.scalar.dma_start(wv_t, wv_d.ap())
                for ko in range(CT):
                    sqx = stA.tile([P, 512], BF16, tag="sqx", name="sqx", bufs=4)
                    nc.vector.tensor_tensor(sqx, xo_blk[:, ko], xo_blk[:, ko],
                                            ALU.mult)
                    nc.tensor.matmul(pmu0[0:1, :], ones_bf, xo_blk[:, ko],
                                     start=(ko == 0), stop=(ko == CT - 1))
                    nc.tensor.matmul(psq0[0:1, :], ones_bf, sqx,
                                     start=(ko == 0), stop=(ko == CT - 1))
                muA = stA.tile([1, 512], F32, tag="muA", name="muA")
                nc.vector.tensor_scalar(muA, pmu0[0:1, :], 1.0 / C, None, ALU.mult)
                varA = stA.tile([1, 512], F32, tag="varA", name="varA")
                nc.vector.tensor_scalar(varA, psq0[0:1, :], 1.0 / C, None, ALU.mult)
                musqA = stA.tile([1, 512], F32, tag="musqA", name="musqA")
                nc.vector.tensor_tensor(musqA, muA, muA, ALU.mult)
                nc.vector.tensor_tensor(varA, varA, musqA, ALU.subtract)
                rstdA = stA.tile([1, 512], F32, tag="rstdA", name="rstdA")
                nc.scalar.activation(rstdA, varA, AF.Sqrt, bias=eps_t[0:1])
                nc.vector.reciprocal_approx_fast(out=rstdA, in_=rstdA)
                murstdA = stA.tile([1, 512], F32, tag="murstdA", name="murstdA")
                nc.vector.tensor_tensor(murstdA, muA, rstdA, ALU.mult)
                # stats_g[.,0] carries rstd/S8: the fp8 qkv psums are in
                # S8-scaled units, one evict multiply undoes both
                rstdA_s = stA.tile([1, 512], F32, tag="rstdAs", name="rstdA_s")
                nc.vector.tensor_scalar(rstdA_s, rstdA, 1.0 / S8, None,
                                        ALU.mult)
                nc.gpsimd.dma_start(_row(stats_loc[0, :]), rstdA_s)
                nc.gpsimd.dma_start(_row(stats_loc[1, :]), murstdA)
            nc.gpsimd.collective_compute(
                "AllGather", ALU.bypass,
                replica_groups=[list(range(R))],
                ins=[stats_loc.opt()], outs=[stats_g.opt()])
            masks_t = wpool.tile([P, P], BF16)
            nc.scalar.dma_start(masks_t, masks_d.ap())

            # =========== Phase B+C pools ===========
            with tc.tile_pool(name="qkvres", bufs=1) as qkvres, \
                 tc.tile_pool(name="xtp", bufs=2) as xtp, \
                 tc.tile_pool(name="reps", bufs=3) as reps, \
                 tc.tile_pool(name="tmps", bufs=4) as tmps, \
                 tc.tile_pool(name="attn", bufs=5) as attnp:

                # persistent qkv (bf16)
                qT = [[qkvres.tile([P, T], BF16, name=f"qT{h}{b}")
                       for b in range(B)] for h in range(HL)]
                kT = [[qkvres.tile([P, T], BF16, name=f"kT{h}{b}")
                       for b in range(B)] for h in range(HL)]
                vsb = [qkvres.tile([P, T // P, HL * D], BF16, name=f"v{b}")
                       for b in range(B)]

                # =========== Phase B: QKV ===========
                # first RAW_BLOCKS blocks evict uncorrected (stats AG not yet
                # done); LN1 correction applied in-place afterwards.
                RAW_BLOCKS = 4
                sv_rep_b = wpool.tile([P, HL * D], BF16)
                nc.gpsimd.dma_start(sv_rep_b, bass.AP(
                    tensor=nsv_d, offset=0, ap=[[0, P], [1, HL * D]]))
                sv_rep = wpool.tile([P, HL * D], F32)
                nc.vector.tensor_copy(sv_rep, sv_rep_b)
                nsq_cb = wpool.tile([P, HL], BF16)
                nc.gpsimd.dma_start(nsq_cb, nsq_d.ap().rearrange("(hl d) -> d hl", d=P))
                nsq_c = wpool.tile([P, HL], F32)
                nc.vector.tensor_copy(nsq_c, nsq_cb)
                nsk_cb = wpool.tile([P, HL], BF16)
                nc.gpsimd.dma_start(nsk_cb, nsk_d.ap().rearrange("(hl d) -> d hl", d=P))
                nsk_c = wpool.tile([P, HL], F32)
                nc.vector.tensor_copy(nsk_c, nsk_cb)
                for b in range(B):
                    for tb in range(NT):
                        j = NT * b + tb
                        t0 = 512 * tb
                        raw = j < RAW_BLOCKS
                        if not raw:
                            # these depend on the stats AllGather — emitting
                            # them for raw blocks would stall the in-order
                            # gpsimd DMA queue on the collective
                            murow_f = reps.tile([1, 512], F32, tag="murowf",
                                                name="murow_f")
                            nc.gpsimd.dma_start(murow_f, _row(stats_g[j, 1, :]))
                            murow = reps.tile([1, 512], BF16, tag="murow",
                                              name="murow")
                            nc.vector.tensor_copy(murow, murow_f)
                            rstd_rep = reps.tile([P, 512], F32, tag="rrep",
                                                 name="rstd_rep")
                            nc.gpsimd.dma_start(rstd_rep, _pbc(stats_g[j, 0, :], 512))

                        pq = [ps() for _ in range(HL)]
                        pk = [ps() for _ in range(HL)]
                        # one bank per 128-token v subtile (start=True clears
                        # the whole bank, chains must not share one)
                        pv = [ps() for _ in range(4)]
                        # one 2MB batched DMA per t-block (per-DMA overhead
                        # dominates many small loads)
                        xt_blk = xtp.tile([P, KP, 2, 512], F8, tag="xt",
                                          name="xt", bufs=3)
                        (nc.sync if j % 2 == 0 else nc.scalar).dma_start(
                            xt_blk, xTb_d.ap()[b, tb])
                        for kp in range(KP):
                            xt = xt_blk[:, kp]          # [P, 2, 512]
                            st_flag = kp == 0
                            lst = raw and kp == KP - 1
                            for hl in range(HL):
                                nc.tensor.matmul(
                                    pq[hl],
                                    wq_t[:, kp, :, hl * D:(hl + 1) * D], xt,
                                    start=st_flag, stop=lst, perf_mode=DR)
                                nc.tensor.matmul(
                                    pk[hl],
                                    wk_t[:, kp, :, hl * D:(hl + 1) * D], xt,
                                    start=st_flag, stop=lst, perf_mode=DR)
                            for ss in range(4):
                                nc.tensor.matmul(
                                    pv[ss][:, 0:256],
                                    xt[:, :, ss * P:(ss + 1) * P],
                                    wv_t[:, kp],
                                    start=st_flag, stop=lst, perf_mode=DR)
                        if raw:
                            for hl in range(HL):
                                nc.vector.tensor_copy(
                                    qT[hl][b][:, t0:t0 + 512], pq[hl])
                                nc.vector.tensor_copy(
                                    kT[hl][b][:, t0:t0 + 512], pk[hl])
                            for ss in range(4):
                                nc.vector.tensor_copy(
                                    vsb[b][:, tb * 4 + ss, :], pv[ss][:, 0:256])
                            continue
                        # rank-1 LN fold: += (-colsum) x murstd  (K=1 matmul)
                        for hl in range(HL):
                            nc.tensor.matmul(
                                pq[hl], nsq_t[0:1, hl * D:(hl + 1) * D], murow,
                                start=False, stop=True)
                            nc.tensor.matmul(
                                pk[hl], nsk_t[0:1, hl * D:(hl + 1) * D], murow,
                                start=False, stop=True)
                        for ss in range(4):
                            nc.tensor.matmul(
                                pv[ss][:, 0:256],
                                murow[0:1, ss * P:(ss + 1) * P], nsv_t,
                                start=False, stop=True)
                        # evictions: single mult by rstd
                        for hl in range(HL):
                            nc.vector.tensor_tensor(
                                qT[hl][b][:, t0:t0 + 512], pq[hl], rstd_rep, ALU.mult)
                            nc.vector.tensor_tensor(
                                kT[hl][b][:, t0:t0 + 512], pk[hl], rstd_rep, ALU.mult)
                        for ss in range(4):
                            si = tb * 4 + ss
                            rstd_c = tmps.tile([P, 1], F32, tag="rc", name="rc")
                            nc.gpsimd.dma_start(
                                rstd_c,
                                stats_g[j, 0, ss * P:(ss + 1) * P]
                                .rearrange("(p o) -> p o", o=1))
                            nc.vector.tensor_scalar(
                                vsb[b][:, si, :], pv[ss][:, 0:256], rstd_c, None,
                                ALU.mult)

                # in-place LN1 fixups for raw blocks (wait on stats AG)
                for j in range(RAW_BLOCKS):
                    fb, ftb = j // NT, j % NT
                    ft0 = 512 * ftb
                    fmur = reps.tile([P, 512], F32, tag="fxmr", name="fx_mur")
                    nc.gpsimd.dma_start(fmur, _pbc(stats_g[j, 1, :], 512))
                    frstd = reps.tile([P, 512], F32, tag="fxrr", name="fx_rstd")
                    nc.gpsimd.dma_start(frstd, _pbc(stats_g[j, 0, :], 512))
                    for hl in range(HL):
                        for (tile_, s_col) in ((qT[hl][fb], nsq_c[:, hl:hl + 1]),
                                               (kT[hl][fb], nsk_c[:, hl:hl + 1])):
                            tmp = tmps.tile([P, 512], F32, tag="fxt", name="fxt")
                            nc.vector.tensor_scalar(tmp, fmur, s_col, None,
                                                    ALU.mult)
                            nc.vector.tensor_tensor(tmp, tile_[:, ft0:ft0 + 512],
                                                    tmp, ALU.add)
                            nc.vector.tensor_tensor(tile_[:, ft0:ft0 + 512], tmp,
                                                    frstd, ALU.mult)
                    for ss in range(4):
                        si = ftb * 4 + ss
                        frc = tmps.tile([P, 1], F32, tag="rc", name="rc")
                        nc.gpsimd.dma_start(
                            frc, stats_g[j, 0, ss * P:(ss + 1) * P]
                            .rearrange("(p o) -> p o", o=1))
                        fmc = tmps.tile([P, 1], F32, tag="mc", name="mc")
                        nc.gpsimd.dma_start(
                            fmc, stats_g[j, 1, ss * P:(ss + 1) * P]
                            .rearrange("(p o) -> p o", o=1))
                        tmpv = tmps.tile([P, HL * D], F32, tag="fxv", name="fxv")
                        nc.vector.tensor_scalar(tmpv, sv_rep, fmc, None, ALU.mult)
                        nc.vector.tensor_tensor(tmpv, vsb[fb][:, si, :], tmpv,
                                                ALU.add)
                        nc.vector.tensor_scalar(vsb[fb][:, si, :], tmpv, frc,
                                                None, ALU.mult)

                # =========== Phase C: attention (head-major) ===========
                # b=1 first: b=0's raw-block LN fixups (DVE) then overlap
                # with (hl0,b1) attention instead of stalling its start.
                for hl in range(HL):
                    for b in (1, 0):
                        for tb in range(NT):
                            t0 = 512 * tb
                            n_s = 4 * (tb + 1)
                            pot = ps()
                            pden = ps()
                            for si in range(n_s):
                                m = si - (n_s - 4)
                                w0 = max(m, 0) * P   # masked-out prefix width
                                pS = ps()
                                nc.tensor.matmul(
                                    pS[:, w0:512],
                                    kT[hl][b][:, si * P:(si + 1) * P],
                                    qT[hl][b][:, t0 + w0:t0 + 512],
                                    start=True, stop=True)
                                pt = attnp.tile([P, 512], BF16, tag="pt", name="pt")
                                nc.scalar.activation(pt[:, w0:512], pS[:, w0:512],
                                                     AF.Exp)
                                if m >= 0:
                                    # on GPSIMD: on the DVE queue this op
                                    # sits behind ~35us of LN fixups and
                                    # head-blocks the whole pot chain
                                    nc.gpsimd.tensor_tensor(
                                        pt[:, w0:w0 + P], pt[:, w0:w0 + P],
                                        masks_t, ALU.mult)
                                nc.tensor.matmul(
                                    pot[:, w0:512],
                                    vsb[b][:, si, hl * D:(hl + 1) * D],
                                    pt[:, w0:512],
                                    start=(si == 0), stop=(si == n_s - 1))
                                nc.tensor.matmul(
                                    pden[0:1, w0:512], ones_bf, pt[:, w0:512],
                                    start=(si == 0), stop=(si == n_s - 1))
                            # normalize: OT / den
                            den_r = attnp.tile([1, 512], F32, tag="dr", name="den_r")
                            nc.vector.reciprocal_approx_fast(out=den_r, in_=pden[0:1, :])
                            den_d = dram.tile([512], F32, tag="den_d", bufs=4,
                                              name="den_d")
                            nc.gpsimd.dma_start(_row(den_d), den_r)
                            den_rep = reps.tile([P, 512], F32, tag="denrep",
                                                name="den_rep")
                            nc.gpsimd.dma_start(den_rep, _pbc(den_d, 512))
                            ot = attnp.tile([P, 512], BF16, tag="ot", name="ot")
                            nc.vector.tensor_tensor(ot, pot, den_rep, ALU.mult)
                            nc.gpsimd.dma_start(a2a_in[hl][NT * b + tb, :, :], ot)
                    # per-head AllToAll fires as soon as head hl is done
                    nc.gpsimd.collective_compute(
                        "AllToAll", ALU.bypass,
                        replica_groups=[list(range(R))],
                        ins=[a2a_in[hl].opt()], outs=[a2a_out[hl].opt()])

            _wpool_cm.__exit__(None, None, None)

            # =========== Phase E: MLP (token-sharded, bf16) ===========
            with tc.tile_pool(name="mlp_x1", bufs=1) as x1p, \
                 tc.tile_pool(name="mlp_sq", bufs=2) as sqp, \
                 tc.tile_pool(name="mlp_x1bf", bufs=1) as x1bfp, \
                 tc.tile_pool(name="mlp_g", bufs=1) as gp, \
                 tc.tile_pool(name="mlp_w1", bufs=2) as w1p, \
                 tc.tile_pool(name="mlp_w2", bufs=2) as w2p, \
                 tc.tile_pool(name="mlp_z", bufs=4) as zp, \
                 tc.tile_pool(name="mlp_out", bufs=2) as outp:

                x1bf = [x1bfp.tile([P, TOK], BF16, name=f"x1bf{i}")
                        for i in range(CT)]
                # pass 1: build x1 tiles (evens first: only need a2a head 0),
                # stats matmuls, bf16 copy. x1 stays RESIDENT in SBUF (no
                # DRAM spill). All DMAs batched. mm1's mg0 even-ct matmuls
                # are emitted between the even and odd builds so PE overlaps
                # the head-1 AllToAll.
                pmu = ps()
                psq = ps()
                order = [2 * i for i in range(CT // 2)] + \
                        [2 * i + 1 for i in range(CT // 2)]
                evens = order[:CT // 2]
                odds = order[CT // 2:]

                # x1 is built IN PLACE over xo_all (residual add) and stays
                # resident in SBUF through mm2 (no DRAM spill/readback)
                x1_all = x1p.tile([P, CT, TOK], F32, name="x1_all")
                nc.sync.dma_start(x1_all, xT_own_d.ap())

                def build_x1(idx, ct):
                    at = x1p.tile([P, TOK], BF16, tag="at", name="at", bufs=3)
                    nc.gpsimd.dma_start(at, a2a_out[ct % 2][ct // 2])
                    nc.vector.tensor_tensor(x1_all[:, ct], x1_all[:, ct],
                                            at, ALU.add)
                    nc.vector.tensor_copy(x1bf[ct], x1_all[:, ct])
                    sq2 = sqp.tile([P, TOK], BF16, tag="sq2", name="sq2")
                    nc.vector.tensor_tensor(sq2, x1bf[ct], x1bf[ct], ALU.mult)
                    nc.tensor.matmul(pmu[0:1, :], ones_bf, x1bf[ct],
                                     start=(idx == 0), stop=(idx == CT - 1))
                    nc.tensor.matmul(psq[0:1, :], ones_bf, sq2,
                                     start=(idx == 0), stop=(idx == CT - 1))

                def mm1_w1_tiles(mg):
                    w1blk = w1p.tile([P, CT, 512], BF16, tag="w1t", name="w1t")
                    nc.sync.dma_start(w1blk, w1_d.ap()[mg])
                    ns1g = zp.tile([1, 512], BF16, tag="ns1g", name="ns1g")
                    nc.gpsimd.dma_start(
                        ns1g, _row(ns1_d.ap()[mg * 512:(mg + 1) * 512]))
                    return w1blk, ns1g

                def mm1_group(mg, blk, pg, group, first):
                    for ci, ct in enumerate(group):
                        st_f = first and ci == 0
                        for ml in range(4):
                            nc.tensor.matmul(
                                pg[ml], blk[:, ct, ml * P:(ml + 1) * P],
                                x1bf[ct], start=st_f, stop=False)

                def mm1_finish(mg, ns1g, pg):
                    for ml in range(4):
                        mt = mg * 4 + ml
                        nc.tensor.matmul(
                            pg[ml], ns1g[0:1, ml * P:(ml + 1) * P],
                            murow2, start=False, stop=True)
                        zt = zp.tile([P, TOK], BF16, tag="zt", name="zt")
                        nc.vector.tensor_tensor(zt, pg[ml], rstd2_rep,
                                                ALU.mult)
                        nc.scalar.activation(gT[:, mt, :], zt,
                                             AF.Gelu_apprx_tanh)

                gT = gp.tile([P, MT, TOK], BF16)
                for idx, ct in enumerate(evens):
                    build_x1(idx, ct)
                # mg0 evens: runs on PE while head-1 A2A is in flight
                # (psum: pmu+psq+4 banks = 6 <= 8, no deadlock)
                wts0, ns1g0 = mm1_w1_tiles(0)
                pg0 = [ps() for _ in range(4)]
                mm1_group(0, wts0, pg0, evens, True)
                for idx, ct in enumerate(odds):
                    build_x1(CT // 2 + idx, ct)
                # finalize stats: mu = pmu/C ; var = psq/C - mu^2
                mu2 = singles.tile([1, TOK], F32)
                nc.vector.tensor_scalar(mu2, pmu[0:1, :], 1.0 / C, None, ALU.mult)
                var2 = singles.tile([1, TOK], F32)
                nc.vector.tensor_scalar(var2, psq[0:1, :], 1.0 / C, None, ALU.mult)
                musq = singles.tile([1, TOK], F32)
                nc.vector.tensor_tensor(musq, mu2, mu2, ALU.mult)
                nc.vector.tensor_tensor(var2, var2, musq, ALU.subtract)
                rstd2 = singles.tile([1, TOK], F32)
                nc.scalar.activation(rstd2, var2, AF.Sqrt, bias=eps_t[0:1])
                nc.vector.reciprocal_approx_fast(out=rstd2, in_=rstd2)
                nc.gpsimd.dma_start(_row(mlp_stat_b[0, :]), mu2)
                nc.gpsimd.dma_start(_row(mlp_stat_b[1, :]), rstd2)
                murow2 = singles.tile([1, TOK], BF16)
                nc.vector.tensor_copy(murow2, mu2)
                rstd2_rep = singles.tile([P, TOK], F32)
                nc.gpsimd.dma_start(rstd2_rep, _pbc(mlp_stat_b[1, :], TOK))

                # mg0 odds + fold + evict
                mm1_group(0, wts0, pg0, odds, False)
                mm1_finish(0, ns1g0, pg0)
                # remaining 15 mgs: 7 pairs + final single
                rest = list(range(1, MG))
                pairs = [rest[i:i + 2] for i in range(0, len(rest), 2)]
                for mgs in pairs:
                    wtss, ns1gs, pgs = {}, {}, {}
                    for mg in mgs:
                        wtss[mg], ns1gs[mg] = mm1_w1_tiles(mg)
                        pgs[mg] = [ps() for _ in range(4)]
                    for group, first in ((evens, True), (odds, False)):
                        for mg in mgs:
                            mm1_group(mg, wtss[mg], pgs[mg], group, first)
                    for mg in mgs:
                        mm1_finish(mg, ns1gs[mg], pgs[mg])

                # matmul2 + residual (x1 resident in SBUF) -> outT
                # w2 streamed in half-co chunks to halve SBUF footprint
                for co in range(CT):
                    w2r_co = w2r_d.ap()[co].rearrange("mo p c -> p mo c")
                    w2t = [w2p.tile([P, MT // 2, P], BF16, tag="w2t",
                                    name="w2t") for _ in range(2)]
                    nc.scalar.dma_start(w2t[0], w2r_co[:, 0:MT // 2])
                    nc.scalar.dma_start(w2t[1], w2r_co[:, MT // 2:MT])
                    po = ps()
                    for mt in range(MT):
                        nc.tensor.matmul(po, w2t[mt // (MT // 2)]
                                         [:, mt % (MT // 2), :], gT[:, mt, :],
                                         start=(mt == 0), stop=(mt == MT - 1))
                    ot2 = outp.tile([P, TOK], F32, tag="ot2", name="ot2")
                    nc.vector.tensor_tensor(ot2, po, x1_all[:, co], ALU.add)
                    nc.scalar.dma_start(out_d.ap()[co * P:(co + 1) * P, :], ot2)

    nc.compile()
    return nc


def _host_prep(x, w_qkv, w1, w2, ln_w):
    x = np.asarray(x, dtype=np.float32)
    w_qkv = np.asarray(w_qkv, dtype=np.float32)
    w1 = np.asarray(w1, dtype=np.float32)
    w2 = np.asarray(w2, dtype=np.float32)
    ln_w = np.asarray(ln_w, dtype=np.float32)

    xT = np.ascontiguousarray(x.transpose(0, 2, 1))            # [B, C, T]
    xT_bf = xT.astype(ml_dtypes.bfloat16)

    def q8(a):
        return np.clip(a, -240, 240).astype(ml_dtypes.float8_e4m3fn)

    # qkv weights quantized to fp8 at scale S8; column sums taken over the
    # QUANTIZED values so the mean-fold cancels exactly
    Wq8 = q8((ln_w[:, None] * w_qkv[:, 0 * C:1 * C]) * (SCALE * S8))
    Wk8 = q8(ln_w[:, None] * w_qkv[:, 1 * C:2 * C] * S8)
    Wv8 = q8(ln_w[:, None] * w_qkv[:, 2 * C:3 * C] * S8)
    nsq_full = -Wq8.astype(np.float64).sum(0).astype(np.float32)
    nsk_full = -Wk8.astype(np.float64).sum(0).astype(np.float32)
    nsv_full = -Wv8.astype(np.float64).sum(0).astype(np.float32)

    def pair_major(w8):
        # [C, n] -> [P, KP, 2, n] (fp8 DoubleRow k-pair layout)
        n = w8.shape[1]
        return np.ascontiguousarray(
            w8.reshape(KP, 2, P, n).transpose(2, 0, 1, 3))

    # xT8: [B, NT, P, KP, 2, 512]
    xT8 = np.ascontiguousarray(
        q8(xT).reshape(B, KP, 2, P, NT, 512).transpose(0, 4, 3, 1, 2, 5))

    w1s = ln_w[:, None] * w1
    # w1 arranged [MG, P, CT*512]: per-mg 2D-contiguous DMA
    w1_bf = np.ascontiguousarray(
        w1s.reshape(CT, P, MG, 512).transpose(2, 1, 0, 3)
    ).reshape(MG, P, CT * 512).astype(ml_dtypes.bfloat16)
    ns1 = -w1s.sum(0, dtype=np.float64).astype(np.float32)
    # w2 reordered: [CT, MT, P(m), P(c)]
    w2r = np.ascontiguousarray(
        w2.reshape(MT, P, CT, P).transpose(2, 0, 1, 3)).astype(ml_dtypes.bfloat16)

    masks = (np.arange(P)[None, :] >= np.arange(P)[:, None]).astype(
        np.float32).astype(ml_dtypes.bfloat16)

    def chan_major(a):
        # [C, N] -> [P, CT*N]: per-partition contiguous rows
        n = a.shape[1]
        return np.ascontiguousarray(
            a.reshape(CT, P, n).transpose(1, 0, 2)).reshape(P, CT * n)

    in_maps = []
    for r in range(R):
        cs = slice(256 * r, 256 * (r + 1))
        b_own, tb_own = r // NT, r % NT
        in_maps.append({
            "xTb": xT8,
            "xT_own": chan_major(np.ascontiguousarray(
                xT[b_own][:, 512 * tb_own: 512 * (tb_own + 1)])),
            "xt_own": chan_major(np.ascontiguousarray(
                xT_bf[b_own][:, 512 * tb_own: 512 * (tb_own + 1)])),
            "wq": pair_major(Wq8[:, cs]),
            "wk": pair_major(Wk8[:, cs]),
            "wv": pair_major(Wv8[:, cs]),
            "nsq": np.ascontiguousarray(nsq_full[cs]).astype(ml_dtypes.bfloat16),
            "nsk": np.ascontiguousarray(nsk_full[cs]).astype(ml_dtypes.bfloat16),
            "nsv": np.ascontiguousarray(nsv_full[cs]).astype(ml_dtypes.bfloat16),
            "w1": w1_bf,
            "ns1": ns1.astype(ml_dtypes.bfloat16),
            "w2r": w2r,
            "masks": masks,
        })
    return in_maps


def get_nc():
    if "nc" not in _CACHE:
        _CACHE["nc"] = _build()
    return _CACHE["nc"]


def run(in_maps, **kw):
    nc = get_nc()
    return run_bass_kernel_spmd(nc, in_maps, core_ids=list(range(R)), **kw)


def kernel(x, w_qkv, w1, w2, ln_w, **kw_unused):
    in_maps = _host_prep(x, w_qkv, w1, w2, ln_w)
    res = run(in_maps)
    out_flat = np.empty((B * T, C), np.float32)
    for r in range(R):
        out_flat[TOK * r: TOK * (r + 1)] = res.results[r]["outT"].T
    return out_flat.reshape(B, T, C)

